# revision 1
# baseline (speedup 1.0000x reference)
"""Self-contained Trainium2 Bass kernel for nn_ActorMpnn (8-core SPMD MPNN), v2.

kernel(**inputs) takes the FULL unsharded inputs and returns the FULL
[B, N, 8] float32 output.

Strategy: 2 cores per graph (B=4, 8 cores). Host-side preprocessing sorts
each graph's nodes by in-degree bucket, splits them into two balanced
half-shards, and lays the edges out in 512-slot tiles (bucket k, m=512//k
nodes per tile) so the min-aggregation becomes regular strided reductions.

v2 edge phase (per ~4096-slot chunk):
  - one HBM row-gather (dma_gather transpose=False, 256B/row descriptors)
    of projected source rows from an HBM table written each layer,
  - one DMA XBAR block transpose (dma_start transpose=True) to column form,
  - fused DVE ops: ea*weac add (STT), per-node r broadcast add (TT per
    uniform-k span), LeakyReLU (STT mult/max),
  - W2 matmuls into PSUM + Act Lrelu+bias drains,
  - DVE reduce-min per uniform-k span into node columns.
Node phase per layer: column projections (18+9 matmuls), one XBAR transpose
+ one DMA to refresh the HBM gather table. Halves exchanged with a 2-core
AllGather through HBM. Final linear + softplus on device; host
inverse-permutes the output.
"""
import sys as _sys
for _p in ('/opt/trn_rl_repo',):
    if _p not in _sys.path:
        _sys.path.insert(0, _p)
import numpy as np
import ml_dtypes


bf16 = ml_dtypes.bfloat16

K_LIST = [4, 6, 8, 10, 12, 14, 16, 18, 20, 22, 24, 28, 32, 40, 48, 64, 96, 128]
M_K = {k: 512 // k for k in K_LIST}
TILE = 512
B, N, E = 4, 8192, 131072
NODE_F, HID = 16, 128


def bucket_of(d):
    for k in K_LIST:
        if k >= d:
            return k
    raise AssertionError(f"degree {d} exceeds max bucket")


def build_layout(edge_index, edge_features):
    """Returns layout dict. Tiles sorted by bucket k; per-core shard data."""
    ea_all = [np.asarray(edge_features[g, :, 0], np.float32) for g in range(B)]
    per_graph = []
    for g in range(B):
        src = np.asarray(edge_index[g, 0], dtype=np.int64)
        dst = np.asarray(edge_index[g, 1], dtype=np.int64)
        deg = np.bincount(dst, minlength=N)
        assert deg.max() <= K_LIST[-1]
        buck = np.array([bucket_of(d) if d > 0 else 0 for d in deg])
        active = np.nonzero(deg > 0)[0]
        order = active[np.lexsort((active, buck[active]))]
        halves = [order[0::2], order[1::2]]
        zeros = np.nonzero(deg == 0)[0]
        zhalves = [zeros[: len(zeros) // 2], zeros[len(zeros) // 2:]]
        eorder = np.argsort(dst, kind="stable")
        starts = np.zeros(N + 1, np.int64)
        starts[1:] = np.cumsum(deg)
        per_graph.append(dict(src=src, dst=dst, deg=deg, buck=buck,
                              halves=halves, zhalves=zhalves,
                              eorder=eorder, starts=starts))

    counts = {k: 0 for k in K_LIST}
    for pg in per_graph:
        for h in range(2):
            b = pg["buck"][pg["halves"][h]]
            for k in K_LIST:
                counts[k] = max(counts[k], int((b == k).sum()))
    N_k = {}
    for k in K_LIST:
        m = M_K[k]
        N_k[k] = int(np.ceil(counts[k] / m) * m) if counts[k] else 0
    n_zero = max(max(len(z) for pg in per_graph for z in pg["zhalves"]), 0)

    C_active = sum(N_k.values())
    C = C_active + n_zero
    C = int(np.ceil(C / 128) * 128)
    T = sum(N_k[k] // M_K[k] for k in K_LIST)
    T = int(np.ceil(T / 16) * 16)            # whole chunks of 16 tiles
    E_pad = T * TILE

    tiles = []
    out_base = 0
    for k in K_LIST:
        m = M_K[k]
        for t in range(N_k[k] // m if N_k[k] else 0):
            tiles.append((k, m, out_base))
            out_base += m
    while len(tiles) < T:
        tiles.append((0, 0, out_base))

    layout = dict(N_k=N_k, C=C, C_active=C_active, T=T, E_pad=E_pad,
                  n_zero=n_zero, tiles=tiles, per_graph=per_graph)

    shards = []
    for g in range(B):
        pg = per_graph[g]
        slotof = np.full(N, -1, np.int64)
        core_nodes = []
        for h in range(2):
            nodes_h = pg["halves"][h]
            b_h = pg["buck"][nodes_h]
            pos_nodes = np.full(C, -1, np.int64)
            p = 0
            for k in K_LIST:
                sel = nodes_h[b_h == k]
                pos_nodes[p:p + len(sel)] = sel
                p += N_k[k]
            zh = pg["zhalves"][h]
            pos_nodes[p:p + len(zh)] = zh
            core_nodes.append(pos_nodes)
            real = pos_nodes >= 0
            slotof[pos_nodes[real]] = h * C + np.nonzero(real)[0]
        assert (slotof[pg["deg"] > 0] >= 0).all()

        for h in range(2):
            pos_nodes = core_nodes[h]
            idx = np.zeros(E_pad, np.int64)
            eav = np.zeros(E_pad, np.float32)
            for t, (k, m, ob) in enumerate(layout["tiles"]):
                es = t * TILE
                if k == 0:
                    idx[es:es + TILE] = h * C
                    continue
                for j in range(m):
                    pos = ob + j
                    node = pos_nodes[pos]
                    if node < 0:
                        idx[es + j * k: es + (j + 1) * k] = h * C
                    else:
                        d = pg["deg"][node]
                        e0 = pg["starts"][node]
                        eids = pg["eorder"][e0:e0 + d]
                        ss = slotof[pg["src"][eids]]
                        assert (ss >= 0).all()
                        idx[es + j * k: es + j * k + d] = ss
                        idx[es + j * k + d: es + (j + 1) * k] = ss[0]
                        eav[es + j * k: es + j * k + d] = ea_all[g][eids]
                        eav[es + j * k + d: es + (j + 1) * k] = ea_all[g][eids[0]]
                tail0 = es + m * k
                if tail0 < es + TILE:
                    idx[tail0: es + TILE] = idx[tail0 - 1]
                    eav[tail0: es + TILE] = eav[tail0 - 1]

            shards.append(dict(g=g, h=h, idx=idx, ea=eav,
                               pos_nodes=pos_nodes, slotof=slotof))
    layout["shards"] = shards

    # uniform-k spans for fused radd/reduce: list of (k, m, tile0, ntiles, ob)
    spans = []
    cur = None
    for t, (k, m, ob) in enumerate(layout["tiles"]):
        if k == 0:
            cur = None
            continue
        if cur is not None and cur[0] == k and cur[2] + cur[3] == t:
            cur[3] += 1
        else:
            cur = [k, m, t, 1, ob]
            spans.append(cur)
    layout["spans"] = [tuple(c) for c in spans]
    return layout


def preprocess(node_features, edge_index, edge_features):
    layout = build_layout(edge_index, edge_features)
    C, E_pad = layout["C"], layout["E_pad"]
    per_core = []
    for ci, sh in enumerate(layout["shards"]):
        g, h = sh["g"], sh["h"]
        idx = sh["idx"].astype(np.int16)
        idx_w = idx.reshape(E_pad // 16, 16).T.copy()
        idx_w = np.vstack([idx_w, idx_w]).copy()
        x0 = np.asarray(node_features[g], np.float32)
        x0_own = np.zeros((NODE_F, C), np.float32)
        pos = sh["pos_nodes"]
        real = pos >= 0
        x0_own[:, np.nonzero(real)[0]] = x0[pos[real]].T
        other = layout["shards"][ci ^ 1]
        x0_full = np.zeros((NODE_F, 2 * C), np.float32)
        x0_full[:, h * C:(h + 1) * C] = x0_own[:, :C]
        opos = other["pos_nodes"]
        oreal = opos >= 0
        x0_full[:, (1 - h) * C + np.nonzero(oreal)[0]] = x0[opos[oreal]].T
        per_core.append(dict(idx_w=idx_w, ea=sh["ea"].astype(bf16).reshape(1, -1),
                             x0_own=x0_own.astype(bf16),
                             x0_full=x0_full.astype(bf16)))
    return layout, per_core


def postprocess(layout, outs):
    res = np.zeros((B, N, 8), np.float32)
    for ci, sh in enumerate(layout["shards"]):
        g = sh["g"]
        pos = sh["pos_nodes"]
        real = pos >= 0
        res[g, pos[real]] = outs[ci][:, np.nonzero(real)[0]].T
    return res


# ======================= device kernel builder =======================
from contextlib import ExitStack
import concourse.bass as bass
import concourse.tile as tile
from concourse import bacc, mybir
from concourse.bass_utils import run_bass_kernel_spmd


FP = mybir.dt.float32
BF = mybir.dt.bfloat16
AF = mybir.ActivationFunctionType
OP = mybir.AluOpType

CHUNK = 8192
CH_TILES = CHUNK // TILE
SIM_COMPAT = False


def build_kernel(C, T, tiles, spans, ca=None, reps=1, stage="full",
                 max_chunks=10**9, sim_compat=None, probe=()):
    if ca is None:
        ca = C
    if sim_compat is None:
        sim_compat = SIM_COMPAT
    SLVL = ["node", "gather", "tpose", "asm", "mm", "nocc", "full"].index(stage)
    E_pad = T * TILE
    n_chunks = E_pad // CHUNK
    nc = bacc.Bacc(num_devices=8)

    # ---------------- DRAM I/O ----------------
    D = {}
    def din(name, shape, dt):
        D[name] = nc.dram_tensor(name, shape, dt, kind="ExternalInput")
    din("idx", [32, E_pad // 16], mybir.dt.int16)
    din("ea", [1, E_pad], BF)
    din("x0o", [NODE_F, C], BF)
    din("x0f", [NODE_F, 2 * C], BF)
    for l in range(3):
        fin = NODE_F if l == 0 else HID
        din(f"wsrc{l}", [fin, HID], BF)
        din(f"wdst{l}", [fin, HID], BF)
        din(f"weac{l}", [HID, 1], FP)
        din(f"b1c{l}", [HID, 1], FP)
        din(f"w2{l}", [HID, HID], BF)
        din(f"b2c{l}", [HID, 1], FP)
    din("lw16", [NODE_F, 8], BF)
    din("lw128", [HID, 8], BF)
    din("lbc", [8, 1], FP)

    out_d = nc.dram_tensor("out", [8, C], FP, kind="ExternalOutput")

    table_d = nc.dram_tensor("ptable", [2 * C, HID], BF)
    ag_in = [nc.dram_tensor(f"agin{l}", [HID, C], BF) for l in range(2)]
    ag_out = [nc.dram_tensor(f"agout{l}", [2, HID, C], BF) for l in range(2)]

    n_stripe = 2 * C // 128

    with tile.TileContext(nc, num_cores=8) as tc:
      with ExitStack() as ctx:
        # ---------------- pools ----------------
        persist = ctx.enter_context(tc.tile_pool(name="persist", bufs=1))
        gpool = ctx.enter_context(tc.tile_pool(name="gpool", bufs=2))
        ebpool = ctx.enter_context(tc.tile_pool(name="ebpool", bufs=1))
        epool = ctx.enter_context(tc.tile_pool(name="epool", bufs=1))
        tpool = ctx.enter_context(tc.tile_pool(name="tpool", bufs=3))
        pp = ctx.enter_context(tc.tile_pool(name="pp", bufs=2, space="PSUM"))

        # ---------------- persistent tiles ----------------
        idx_t = persist.tile([128, E_pad // 16], mybir.dt.int16)
        nc.vector.memset(idx_t[:], 0)
        nc.sync.dma_start(idx_t[0:32, :], D["idx"].ap())

        x0o_t = persist.tile([NODE_F, C], BF)
        nc.sync.dma_start(x0o_t[:], D["x0o"].ap())

        W = {}
        for l in range(3):
            fin = NODE_F if l == 0 else HID
            for nm, sh, dt in [(f"wsrc{l}", [fin, HID], BF),
                               (f"wdst{l}", [fin, HID], BF),
                               (f"weac{l}", [HID, 1], FP),
                               (f"b1c{l}", [HID, 1], FP),
                               (f"w2{l}", [HID, HID], BF),
                               (f"b2c{l}", [HID, 1], FP)]:
                W[nm] = persist.tile(sh, dt, name=nm, tag=nm)
                nc.sync.dma_start(W[nm][:], D[nm].ap())
        for nm, sh, dt in [("lw16", [NODE_F, 8], BF), ("lw128", [HID, 8], BF),
                           ("lbc", [8, 1], FP)]:
            W[nm] = persist.tile(sh, dt, name=nm, tag=nm)
            nc.sync.dma_start(W[nm][:], D[nm].ap())

        x_own = persist.tile([HID, C], BF)
        x_full = persist.tile([HID, 2 * C], BF)
        r_cols = persist.tile([HID, C], BF)
        x3_own = persist.tile([HID, C], BF)
        pcols = persist.tile([128, 2 * C], BF)
        prows = persist.tile([128, n_stripe // 2, 128], BF)

        def node_proj(lhsT, rhs_t, ncols, drain_bias, out_cols,
                      rhs_dram=None, rhs_rows=0):
            if drain_bias is None:
                drain_bias = 0.0
            for j0 in range(0, ncols, 2048):
                wb = min(2048, ncols - j0)
                if rhs_dram is not None:
                    xstg = gpool.tile([NODE_F, 2048], BF, tag="xstg",
                                      name="xstgt")
                    nc.sync.dma_start(xstg[:, :wb],
                                      rhs_dram.ap()[:, j0:j0 + wb])
                    src = xstg
                else:
                    src = None
                pb = pp.tile([128, 2048], FP, tag="ps", name="pst")
                for q0 in range(0, wb, 512):
                    w5 = min(512, wb - q0)
                    mv = (src[:, q0:q0 + w5] if src is not None
                          else rhs_t[:, j0 + q0:j0 + q0 + w5])
                    nc.tensor.matmul(pb[:, q0:q0 + w5], lhsT, mv,
                                     start=True, stop=True)
                nc.scalar.activation(out_cols[:, j0:j0 + wb], pb[:, :wb],
                                     AF.Identity, bias=drain_bias, scale=1.0)

        def leaky_dve(ap_):
            nc.vector.scalar_tensor_tensor(ap_, ap_, 0.01, ap_,
                                           op0=OP.mult, op1=OP.max)

        for rep in range(reps):
            for l in range(3):
                # ---- node phase ----
                # psrc columns -> rows -> HBM table
                if l == 0:
                    node_proj(W[f"wsrc{l}"][:], None, 2 * C, None, pcols,
                              rhs_dram=D["x0f"])
                    # r for layer 0 (layers 1,2 are projected pre-exchange
                    # at the previous layer's tail)
                    node_proj(W["wdst0"][:], x0o_t[:], C, W["b1c0"][:], r_cols)
                else:
                    node_proj(W[f"wsrc{l}"][:], x_full[:], 2 * C, None, pcols)
                for th in range(2):
                    nc.sync.dma_start(prows[:], pcols[:, th * C:(th + 1) * C],
                                      transpose=True)
                    tsl = table_d.ap()[th * C:(th + 1) * C]
                    nc.sync.dma_start(
                        tsl.rearrange("(b p) f -> p b f", p=128), prows[:])

                x_dst = x3_own if l == 2 else x_own
                nc.vector.memset(x_dst[:], 1.0e30)

                # ---- edge phase ----
                if SLVL >= 1:
                  for ch in range(min(n_chunks, max_chunks)):
                    e0 = ch * CHUNK
                    ch_tiles = tiles[ch * CH_TILES:(ch + 1) * CH_TILES]
                    if all(k == 0 for (k, m, ob) in ch_tiles):
                        continue
                    G = gpool.tile([128, CHUNK // 128, 128], BF, tag="G")
                    for g2 in range(CHUNK // 4096):
                        ge = e0 + g2 * 4096
                        nc.gpsimd.dma_gather(
                            out_ap=G[:, g2 * 32:(g2 + 1) * 32, :],
                            in_ap=table_d.ap(),
                            idxs_ap=idx_t[:, ge // 16:ge // 16 + 256],
                            num_idxs=4096, num_idxs_reg=4096, elem_size=128,
                            transpose=False, single_packet=False)
                    if SLVL < 2:
                        continue
                    pass
                    if 'nodmat' in probe:
                        A = G[:].rearrange("p a b -> p (a b)")
                    else:
                        Tt = tpool.tile([128, CHUNK // 128, 128], BF, tag="T")
                        nc.sync.dma_start(Tt[:],
                                          G[:].rearrange("p a b -> p (a b)"),
                                          transpose=True)
                        A = Tt[:].rearrange("p a b -> p (a b)")
                    if SLVL < 3:
                        continue
                    if 'noeab' not in probe:
                        eab = ebpool.tile([128, CHUNK], BF, tag="eab")
                        ea_sl = D["ea"].ap()[:, e0:e0 + CHUNK]
                        ea_bc = bass.AP(tensor=ea_sl.tensor, offset=ea_sl.offset,
                                        ap=[[0, 128], [1, CHUNK]])
                        nc.sync.dma_start(eab[:], ea_bc)
                        nc.vector.scalar_tensor_tensor(
                            A, eab[:], W[f"weac{l}"][:], A,
                            op0=OP.mult, op1=OP.add)
                    # radd per uniform-k piece within this chunk (4D APs,
                    # skipping per-tile filler tails)
                    ch_t0 = ch * CH_TILES
                    for (k, m, t0, nt, ob) in spans:
                        ct0 = max(t0, ch_t0)
                        ct1 = min(t0 + nt, ch_t0 + CH_TILES)
                        if ct0 >= ct1:
                            continue
                        ntt = ct1 - ct0
                        nlo = ob + (ct0 - t0) * m
                        r_sl = r_cols[:, nlo:nlo + ntt * m]
                        r4 = r_sl.rearrange("p (t n) -> p t n", n=m)
                        r_bc = bass.AP(tensor=r4.tensor, offset=r4.offset,
                                       ap=[list(r4.ap[0]), list(r4.ap[1]),
                                           list(r4.ap[2]), [0, k]])
                        off = (ct0 - ch_t0) * TILE
                        a4 = A[:, off:off + ntt * TILE].rearrange(
                            "p (t u) -> p t u", u=TILE)[:, :, 0:m * k].rearrange(
                            "p t (n k) -> p t n k", k=k)
                        if 'noradd' not in probe:
                            nc.vector.tensor_tensor(a4, a4, r_bc, op=OP.add)
                    # leaky1 (Act on HW; DVE in sim_compat)
                    if 'noleaky' in probe:
                        pass
                    elif sim_compat:
                        leaky_dve(A)
                    else:
                        nc.scalar.activation(A, A, AF.Lrelu, bias=0.0,
                                             scale=1.0, alpha=0.01)
                    if SLVL < 4:
                        continue
                    # W2 matmuls into [128,2048] psum tiles; reduce-min reads
                    # PSUM directly (bias+leaky commute with min -> applied
                    # once per layer after the chunk loop)
                    for j in range(CHUNK // 2048):
                        blk_t0 = ch_t0 + j * 4
                        blk_tiles = tiles[blk_t0:blk_t0 + 4]
                        if all(kk == 0 for (kk, mm_, obb) in blk_tiles):
                            continue
                        mb = pp.tile([128, 2048], FP, tag="ps", name="mbt")
                        for q in range(4):
                            if blk_tiles[q][0] == 0:
                                continue
                            if 'mm1' in probe and q > 0:
                                continue
                            c0 = j * 2048 + q * 512
                            nc.tensor.matmul(mb[:, q * 512:(q + 1) * 512],
                                             W[f"w2{l}"][:], A[:, c0:c0 + 512],
                                             start=True, stop=True)
                        for (k, m, t0, nt, ob) in spans:
                            ct0 = max(t0, blk_t0)
                            ct1 = min(t0 + nt, blk_t0 + 4)
                            if ct0 >= ct1:
                                continue
                            ntt = ct1 - ct0
                            nlo = ob + (ct0 - t0) * m
                            off = (ct0 - blk_t0) * TILE
                            s4 = mb[:, off:off + ntt * TILE].rearrange(
                                "p (t u) -> p t u", u=TILE)[:, :, 0:m * k].rearrange(
                                "p t (n k) -> p t n k", k=k)
                            o3 = x_dst[:, nlo:nlo + ntt * m].rearrange(
                                "p (t n) -> p t n", n=m)
                            if 'noreduce' not in probe:
                                nc.vector.tensor_reduce(
                                    o3, s4, axis=mybir.AxisListType.X,
                                    op=OP.min)

                if SLVL >= 4:
                    if sim_compat:
                        nc.scalar.activation(x_dst[:, 0:ca], x_dst[:, 0:ca],
                                             AF.Identity, bias=W[f"b2c{l}"][:],
                                             scale=1.0)
                        leaky_dve(x_dst[:, 0:ca])
                    else:
                        nc.scalar.activation(x_dst[:, 0:ca], x_dst[:, 0:ca],
                                             AF.Lrelu, bias=W[f"b2c{l}"][:],
                                             scale=1.0, alpha=0.01)
                    if ca < C:
                        nc.vector.memset(x_dst[:, ca:C], 0.0)
                elif SLVL >= 1:
                    nc.vector.memset(x_dst[:], 0.0)

                # ---- exchange (layers 0,1) ----
                if l < 2:
                    # r for the NEXT layer needs only our own x: do it before
                    # the all-engine barriers so it overlaps the exchange
                    node_proj(W[f"wdst{l + 1}"][:], x_dst[:], C,
                              W[f"b1c{l + 1}"][:], r_cols)
                    if SLVL >= 6:
                        nc.sync.dma_start(ag_in[l].ap(), x_own[:])
                        tc.strict_bb_all_engine_barrier()
                        nc.gpsimd.collective_compute(
                            "AllGather", OP.bypass,
                            replica_groups=[[0, 1], [2, 3], [4, 5], [6, 7]],
                            ins=[ag_in[l].ap()], outs=[ag_out[l].ap()])
                        tc.strict_bb_all_engine_barrier()
                        nc.sync.dma_start(
                            x_full[:].rearrange("h (g c) -> h g c", g=2),
                            ag_out[l].ap().rearrange("g h c -> h g c"))
                    else:
                        nc.vector.tensor_copy(x_full[:, 0:C], x_own[:])
                        nc.vector.tensor_copy(x_full[:, C:2 * C], x_own[:])

            # ---- final linear + softplus ----
            for j0 in range(0, C, 1024):
                wb = min(1024, C - j0)
                pb = pp.tile([128, 2048], FP, tag="ps", name="fint")
                for q0 in range(0, wb, 512):
                    w5 = min(512, wb - q0)
                    nc.tensor.matmul(pb[0:8, q0:q0 + w5], W["lw16"][:],
                                     x0o_t[:, j0 + q0:j0 + q0 + w5],
                                     start=True, stop=False)
                    nc.tensor.matmul(pb[0:8, q0:q0 + w5], W["lw128"][:],
                                     x3_own[:, j0 + q0:j0 + q0 + w5],
                                     start=False, stop=True)
                ex = epool.tile([8, 1024], FP, tag="fex", name="fext")
                nc.scalar.activation(ex[:, :wb], pb[0:8, :wb], AF.Exp,
                                     bias=W["lbc"][:], scale=1.0)
                oo = epool.tile([8, 1024], FP, tag="fo", name="fot")
                nc.scalar.activation(oo[:, :wb], ex[:, :wb], AF.Ln,
                                     bias=1.0, scale=1.0)
                nc.sync.dma_start(out_d.ap()[:, j0:j0 + wb], oo[:, :wb])

    nc.finalize()
    return nc


def make_in_maps(layout, per_core, inputs):
    maps = []
    for ci in range(8):
        pc = per_core[ci]
        m = dict(idx=pc["idx_w"], ea=pc["ea"],
                 x0o=pc["x0_own"], x0f=pc["x0_full"])
        for l in range(3):
            fin = NODE_F if l == 0 else HID
            w1 = np.asarray(inputs[f"c{l+1}_w1"], np.float32)
            m[f"wdst{l}"] = w1[:fin].astype(bf16)
            m[f"wsrc{l}"] = w1[fin:2 * fin].astype(bf16)
            m[f"weac{l}"] = w1[2 * fin].reshape(HID, 1).astype(np.float32)
            m[f"b1c{l}"] = np.asarray(inputs[f"c{l+1}_b1"], np.float32).reshape(HID, 1)
            m[f"w2{l}"] = np.asarray(inputs[f"c{l+1}_w2"], np.float32).astype(bf16)
            m[f"b2c{l}"] = np.asarray(inputs[f"c{l+1}_b2"], np.float32).reshape(HID, 1)
        lw = np.asarray(inputs["lin_w"], np.float32)
        m["lw16"] = lw[:NODE_F].astype(bf16)
        m["lw128"] = lw[NODE_F:].astype(bf16)
        m["lbc"] = np.asarray(inputs["lin_b"], np.float32).reshape(8, 1)
        maps.append(m)
    return maps


_CACHE = {}


def kernel(node_features, edge_index, edge_features, **weights):
    inputs = dict(weights)
    layout, per_core = preprocess(node_features, edge_index, edge_features)
    key = (layout["C"], layout["T"])
    if key not in _CACHE:
        _CACHE[key] = build_kernel(layout["C"], layout["T"], layout["tiles"],
                                   layout["spans"], ca=layout["C_active"],
                                   reps=1, stage="full")
    nc = _CACHE[key]
    in_maps = make_in_maps(layout, per_core, inputs)
    res = run_bass_kernel_spmd(nc, in_maps, core_ids=list(range(8)))
    outs = [np.asarray(res.results[c]["out"]) for c in range(8)]
    return postprocess(layout, outs).astype(np.float32)

